# revision 3
# baseline (speedup 1.0000x reference)
"""CostVolume2D Trainium2 kernel, v4.

out[b, d, h, w] = mean_c l[b,c,h,w] * r[b,c,h, w - (d - maxd)]   (r zero padded)

Per (b, h): the 97 disparity planes are diagonals of banded gram blocks
G_q[i, jc] = sum_c l[c, 128q+i] r_pad[c, 128q+jc]; row i needs jc in [i, i+97).

Everything here is shaped by two HW observations: (1) each of the 16 SDMA
engines costs ~100ns per packet and a packet is one descriptor (contiguous
run), so descriptors must be multi-KB to hit line rate; (2) per-engine
streaming caps at ~16 GB/s for HBM reads and ~20 GB/s for writes, so total
HBM bytes are the wall (~33.5 MB/core here = reads 16.8 + writes 16.8):
  * Input DRAM layout [b, hh, c, hpair, 1024] (l row | r row, unpadded)
    makes per-channel load runs 8*1024*2 = 16KB; 4 load DMAs per tile
    split across the two HWDGE rings (sync + scalar) -- two rings of
    outstanding read descriptors measurably raise the per-engine read rate.
  * No r padding: edge matmuls read adjacent-l / uninitialized-tail
    garbage, and the host zeroes the affected output positions (they are
    exactly 0 in the reference, which zero-pads r).
  * Gram tile: 4 h-rows (j) share one [128, 3584] tile with columns
    chunk-interleaved: col = 512*c + 128*j + 32*q + e, where jc = 32*c + e
    indexes the 224 gram columns of block q. A 32-row group s then needs
    exactly cols [512s, 512s+2048): 4KB contiguous store runs at 1.32x
    write amplification (128-col window per 97 needed) -> 128 descriptors
    per store DMA, 128 store DMAs alternating sync (HWDGE) / gpsimd
    (SWDGE) so the two descriptor-generation paths run in parallel.
  * All 4 matmuls are N=224 into one 2-bank PSUM tile at 256-col offsets;
    one zip-copy AP per engine evicts everything for an h: DVE takes
    chunks 0..2, scalar (Act) takes 3..6, both writing 64B bursts.
  * Host pre-divides l by C (exact) and unshards with a strided diagonal
    view + flip/transpose + edge mask (pure layout glue).
"""

import sys

try:
    import concourse  # noqa: F401
except ImportError:
    sys.path.insert(0, "/opt/trn_rl_repo")

import numpy as np

from concourse import bass, mybir
from concourse import tile
from concourse.ap import AP
from concourse.bass_utils import run_bass_kernel_spmd

F32 = mybir.dt.float32
F16 = mybir.dt.float16
NP16 = np.float16

# Problem dims (hardcoded per spec)
B, C, H, W = 4, 64, 256, 512
MAXD = 48
D = 2 * MAXD + 1          # 97 disparity planes
NCORES = 8
HS = H // NCORES          # 32 h-rows per core

WB = 128                  # w-block (gram rows per block)
NQ = W // WB              # 4 w-blocks
GW = WB + 2 * MAXD        # 224 gram jcols per block
LRW = 2 * W               # 1024: (l | r) row width, no pads
HGRP = 16                 # h-rows loaded per lr tile
NHQ = 4                   # h-rows (j) sharing one gram tile
CH = 32                   # interleave chunk (jcols per chunk)
NC_ = GW // CH            # 7 chunks per q-block
GTW = NC_ * NHQ * NQ * CH  # 3584 gram tile width
SG = 32                   # store row-group size
NSG = WB // SG            # 4 groups
SRUN = 4 * NHQ * NQ * CH  # 2048 elems per store row (4 chunks x 4j x 4q x 32)

LAST_RESULTS = None
_NC_CACHE = {}


def _build_nc(b_n=B, hs=HS, split_waits=True):
    nc = bass.Bass()
    lr_in = nc.dram_tensor(
        "lr", [b_n, 2, C, hs // 2, LRW], F16, kind="ExternalInput"
    )
    o_out = nc.dram_tensor(
        "o", [b_n, hs // NHQ, NSG, SG, SRUN], F16, kind="ExternalOutput"
    )

    nh4 = HGRP // 2           # h-pairs per lr tile
    # +48 garbage columns so q=3's rhs window of the last h-row stays in
    # bounds; the host zeroes every output position whose r-shift ran off
    # the edge (they are exactly 0 in the reference), so the garbage gram
    # columns those reads produce are never observed.
    lrw = nh4 * LRW + MAXD    # free width of lr tile

    with tile.TileContext(nc) as tc:
        with (
            tc.tile_pool(name="lrpool", bufs=6) as lrp,
            tc.tile_pool(name="gpool", bufs=1) as gp,
            tc.tile_pool(name="ppool", bufs=4, space="PSUM") as pp,
        ):
            g_tiles = [gp.tile([128, GTW], F16, name=f"g{i}") for i in range(8)]
            gi = 0
            st = 0
            for b in range(b_n):
                for hg in range(hs // HGRP):
                    lr_t = lrp.tile([128, lrw], F16, name="lr_t")
                    # partitions = (hh in 2) x (c in 64); free = (h4, w_lr)
                    nc.vector.memset(lr_t[:, nh4 * LRW:], 0.0)
                    ld_engs = (nc.sync, nc.scalar)
                    for hh in range(2):
                        for ch in range(2):   # split for deeper read pipe
                            lr_src = AP(
                                lr_in,
                                (((b * 2 + hh) * C + 32 * ch) * (hs // 2)
                                 + hg * nh4) * LRW,
                                [((hs // 2) * LRW, 32),
                                 (nh4 * LRW // 2, 2), (1, nh4 * LRW // 2)],
                            )
                            eng = ld_engs[(hh + ch) % 2]
                            eng.dma_start(
                                out=lr_t[64 * hh + 32 * ch:
                                         64 * hh + 32 * ch + 32,
                                         :nh4 * LRW],
                                in_=lr_src,
                            )
                    for quad in range(HGRP // NHQ):
                        g = g_tiles[gi % 8]
                        gi += 1
                        ga = g[:, :]
                        for jj in range(2):    # h-pair within quad
                            h4 = 2 * quad + jj
                            for hh in range(2):
                                j = 2 * jj + hh
                                psl = lr_t[64 * hh:64 * hh + 64, :]
                                p = pp.tile([128, 1024], F32, name="p")
                                for q in range(NQ):
                                    lhsT = psl[
                                        :, h4 * LRW + WB * q:
                                        h4 * LRW + WB * q + WB]
                                    rhs = psl[
                                        :, h4 * LRW + W - MAXD + WB * q:
                                        h4 * LRW + W - MAXD + WB * q + GW]
                                    nc.tensor.matmul(
                                        p[:, 256 * q:256 * q + GW],
                                        lhsT, rhs, start=True, stop=True,
                                    )
                                # zip-evict: out col 512c+128j+32q+e from
                                # psum col 256q+32c+e; DVE c<3, Act c>=3
                                pa = p[:, :]
                                for op, c0, nch in (
                                    (nc.vector.tensor_copy, 0, 3),
                                    (nc.scalar.copy, 3, 4),
                                ):
                                    op(
                                        AP(ga.tensor,
                                           ga.offset + 512 * c0 + 128 * j,
                                           [(GTW, 128), (512, nch),
                                            (32, NQ), (1, CH)]),
                                        AP(pa.tensor,
                                           pa.offset + 32 * c0,
                                           [(1024, 128), (32, nch),
                                            (256, NQ), (1, CH)]),
                                    )
                        # 4 store DMAs per quad, one per 32-row group s:
                        # rows [32s,32s+32) x cols [512s, 512s+2048)
                        for s in range(NSG):
                            src = AP(
                                ga.tensor,
                                ga.offset + (s * SG) * GTW + s * 16 * CH,
                                [(GTW, SG), (1, SRUN)],
                            )
                            dst = AP(
                                o_out,
                                (((b * hs) // NHQ + 4 * hg + quad) * NSG + s)
                                * SG * SRUN,
                                [(SRUN, SG), (1, SRUN)],
                            )
                            eng = nc.gpsimd if (st % 2) else nc.sync
                            st += 1
                            eng.dma_start(out=dst, in_=src)
    if split_waits:
        _split_multi_waits(nc)
    return nc


def _split_multi_waits(nc):
    """The 64-byte TPB instruction encoding holds a single semaphore wait;
    walrus codegen rejects instructions whose sync_info carries more. Hoist
    all but one wait onto standalone InstEventSemaphore instructions placed
    immediately before, on the same engine (FIFO order preserves semantics).
    """
    for bb in nc.main_func.blocks:
        new_list = []
        changed = False
        for ins in bb.instructions:
            si = ins.sync_info
            if si is not None and len(si.on_wait) > 1:
                for w in list(si.on_wait)[:-1]:
                    ev = mybir.InstEventSemaphore(
                        name=nc.get_next_instruction_name(),
                        engine=ins.engine,
                        ins=[],
                        outs=[],
                        sync_info=mybir.SyncInfo(on_wait=[w], on_update=[]),
                    )
                    new_list.append(ev)
                ins.sync_info = mybir.SyncInfo(
                    on_wait=[list(si.on_wait)[-1]], on_update=list(si.on_update)
                )
                changed = True
            new_list.append(ins)
        if changed:
            bb.instructions = new_list


def _get_nc(key=(B, HS), split_waits=True):
    if (key, split_waits) not in _NC_CACHE:
        _NC_CACHE[(key, split_waits)] = _build_nc(*key, split_waits=split_waits)
    return _NC_CACHE[(key, split_waits)]


def _host_prep(l_fmap, r_fmap):
    """lr[b, hh, c, hpair, :] = [l row | r row] for h = 2*hpair + hh."""
    l = np.asarray(l_fmap, dtype=np.float32) * np.float32(1.0 / C)
    r = np.asarray(r_fmap, dtype=np.float32)
    lr = np.empty((B, 2, C, H // 2, LRW), dtype=NP16)
    lv = l.reshape(B, C, H // 2, 2, W)
    rv = r.reshape(B, C, H // 2, 2, W)
    for hh in range(2):
        lr[:, hh, :, :, :W] = lv[:, :, :, hh, :]
        lr[:, hh, :, :, W:] = rv[:, :, :, hh, :]
    return lr


def _unshard(res_list):
    out = np.empty((B, D, H, W), dtype=np.float32)
    for k in range(NCORES):
        o = np.asarray(res_list[k]).view(NP16)
        # o: [B, 8quad, 4s, 32i_lo, 2048m], m = ((cr*4+j)*4+q)*CH+e
        # value = G_q^(4quad+j)[32s+i_lo, jc=32s+CH*cr+e]; k' = jc-i in [0,97)
        o8 = o.reshape(B, HS // NHQ, NSG, SG, WB // CH, NHQ, NQ, CH)
        # -> (b, quad, s, i_lo, j, q, cr, e) -> kk = 16cr+e
        o9 = o8.transpose(0, 1, 2, 5, 6, 3, 4, 7)  # b,quad,s,j,q,i_lo,cr,e
        o9 = np.ascontiguousarray(o9).reshape(
            B, HS // NHQ, NSG, NHQ, NQ, SG, WB
        )
        sb, sq_, ss, sj, sq2, si, skk = o9.strides
        diag = np.lib.stride_tricks.as_strided(
            o9, shape=(B, HS // NHQ, NSG, NHQ, NQ, SG, D),
            strides=(sb, sq_, ss, sj, sq2, si + skk, skk),
        )
        # out[b, 96-k', 4quad+j, 128q+32s+i_lo] = diag[b,quad,s,j,q,i_lo,k']
        tmp = np.flip(
            diag.transpose(0, 6, 1, 3, 4, 2, 5), axis=1
        ).astype(np.float32)  # [B, k', quad, j, q, s, i_lo]
        out[:, :, k * HS:(k + 1) * HS, :] = tmp.reshape(B, D, HS, W)
    # r is unpadded on device: gram columns whose r-shift ran off the edge
    # hold garbage; the reference value there is exactly 0 (zero padding).
    di = np.arange(D)[:, None]
    wi = np.arange(W)[None, :]
    invalid = ~((wi >= di - MAXD) & (wi < W - MAXD + di))
    np.copyto(out, 0.0, where=invalid[None, :, None, :])
    return out


def _install_ntff_hook_shim(so_path="/opt/axon/libaxon_pjrt.so"):
    """Provide antenv.axon_hooks.get_axon_ntff_profile_hook via ctypes when
    the image's antenv lacks it (mirrors trn_agent_boot's slim hook)."""
    import types
    import ctypes
    import contextlib

    try:
        from antenv.axon_hooks import get_axon_ntff_profile_hook  # noqa: F401
        return
    except ImportError:
        pass

    lib = ctypes.CDLL(so_path)
    if not hasattr(lib, "axon_start_nrt_profile"):
        return
    lib.axon_start_nrt_profile.argtypes = [
        ctypes.POINTER(ctypes.c_int64), ctypes.c_size_t,
    ]
    lib.axon_start_nrt_profile.restype = ctypes.c_int64
    lib.axon_stop_nrt_profile.argtypes = [ctypes.c_char_p]
    lib.axon_stop_nrt_profile.restype = ctypes.c_int64

    @contextlib.contextmanager
    def _hook(output_dir, device_ids):
        import jax
        jax.devices()
        if device_ids:
            ids = (ctypes.c_int64 * len(device_ids))(*device_ids)
            rc = lib.axon_start_nrt_profile(ids, len(device_ids))
        else:
            rc = lib.axon_start_nrt_profile(None, 0)
        if rc != 0:
            raise RuntimeError(f"axon_start_nrt_profile rc={rc}")
        try:
            yield
        finally:
            n = lib.axon_stop_nrt_profile(str(output_dir).encode())
            print(f"ntff profile: {n} file(s) written to {output_dir}",
                  file=sys.stderr)

    import antenv
    mod = types.ModuleType("antenv.axon_hooks")
    mod.get_axon_ntff_profile_hook = lambda: _hook
    mod.set_axon_ntff_profile_hook = lambda h: None
    sys.modules["antenv.axon_hooks"] = mod
    antenv.axon_hooks = mod


def kernel(l_fmap, r_fmap, max_disp):
    global LAST_RESULTS
    assert int(max_disp) == MAXD
    lr = _host_prep(l_fmap, r_fmap)
    assert lr.shape == (B, 2, C, H // 2, LRW)

    nc = _get_nc()
    in_maps = []
    for k in range(NCORES):
        sl = slice(k * (HS // 2), (k + 1) * (HS // 2))
        in_maps.append({"lr": np.ascontiguousarray(lr[:, :, :, sl, :])})

    import os
    trace = bool(int(os.environ.get("CV_TRACE", "0")))
    if trace:
        _install_ntff_hook_shim()
    res = run_bass_kernel_spmd(nc, in_maps, list(range(NCORES)), trace=trace)
    LAST_RESULTS = res
    return _unshard([res.results[k]["o"] for k in range(NCORES)])
